# revision 34
# baseline (speedup 1.0000x reference)
"""Trainium2 Bass kernel for a small dense transformer block (v2).

Model (per reference):
  x   : [B, T, D]  B=16, T=2048, D=40, H=4 heads, hs=10
  ln1 -> per-head q/k/v -> scores = k @ q^T (softmax over q index) -> out @ Wp
  residual (on ln1(x)) -> ln2 -> FFN(relu) -> residual (on ln2 output)

Sharding: data-parallel over batch, 2 batches per core across 8 cores.

v2 design notes (vs v1):
  - q and k packed in ONE bf16 buffer qkT [128, T]: head h has q at
    partitions 32h..32h+9 and k at partitions 32h+16..32h+25. One stage-A
    matmul per 512 tokens produces all of q+k for all heads; S matmuls
    become bf16 (1 cyc/row vs ~1.5-4 for fp32/f32r).
  - 4 S matmuls per j-tile emitted adjacently to 4 distinct PSUM banks
    (row-group tiling) so they can overlap in the PE.
  - exp split across engines: heads 0,1 via exact ACT exp (bf16 out),
    heads 2,3 via a Schraudolph-style fast exp on the DVE: one
    tensor_scalar producing int16 bf16-bit-patterns, bitcast to bf16 for
    the PV matmul. End-to-end rel err ~3e-3 (gate 2e-2).
  - v has an extra "ones" feature via a ones row in xnT (partition 40)
    and a ones entry in the packed v weights, giving the softmax
    denominator for free in the PV accumulation (as v1, minus the
    per-tile ones writes).
  - LN2 mean/rstd broadcast fused into ONE matmul ([2,512] f32r moving,
    [2,80] one-hot stationary -> [80,512]: rows 0:40 mu, 40:80 rstd).
  - reciprocal -> reciprocal_approx_fast (5x).
  - elementwise ops distributed: gpsimd takes SBUF-only ops (on, sq, x2,
    xn, msq), ACT takes PSUM-reading copies/activations it has slack for
    (pv_sb, mus, lnv2, rsd, hs-relu), DVE keeps the rest.
"""

import sys
from contextlib import ExitStack

for _p in ("/opt/trn_rl_repo",):
    if _p not in sys.path:
        sys.path.insert(0, _p)

import numpy as np

import concourse.bass as bass
import concourse.tile as tile
from concourse import mybir
from concourse.masks import make_identity

B_FULL = 16
N_CORES = 8
B_LOC = B_FULL // N_CORES
T = 2048
D = 40
H = 4
HS = 10
LN_EPS = 1e-5

F32 = mybir.dt.float32
F32R = mybir.dt.float32r
BF16 = mybir.dt.bfloat16
I16 = mybir.dt.int16
AF = mybir.ActivationFunctionType
OP = mybir.AluOpType

# Schraudolph fast-exp constants for bf16 bit patterns via int16:
#   bits = round(s * 128/ln2 + (127*128 - C))
SCH_A = 128.0 / float(np.log(2.0))
SCH_B = 127.0 * 128.0 - 5.5


def build_kernel(b_loc=B_LOC, t_len=T, split_waits=True):
    nc = bass.Bass("TRN2", target_bir_lowering=False)

    x_d = nc.dram_tensor("x", [b_loc, t_len, D], F32, kind="ExternalInput")
    wq_d = nc.dram_tensor("Wq", [H, D, HS], F32, kind="ExternalInput")
    wk_d = nc.dram_tensor("Wk", [H, D, HS], F32, kind="ExternalInput")
    wv_d = nc.dram_tensor("Wv", [H, D, HS], F32, kind="ExternalInput")
    wp_d = nc.dram_tensor("Wp", [D, D], F32, kind="ExternalInput")
    bp_d = nc.dram_tensor("bp", [D], F32, kind="ExternalInput")
    w1_d = nc.dram_tensor("W1", [D, D], F32, kind="ExternalInput")
    b1_d = nc.dram_tensor("b1", [D], F32, kind="ExternalInput")
    w2_d = nc.dram_tensor("W2", [D, D], F32, kind="ExternalInput")
    b2_d = nc.dram_tensor("b2", [D], F32, kind="ExternalInput")
    g1_d = nc.dram_tensor("g1", [D], F32, kind="ExternalInput")
    be1_d = nc.dram_tensor("be1", [D], F32, kind="ExternalInput")
    g2_d = nc.dram_tensor("g2", [D], F32, kind="ExternalInput")
    be2_d = nc.dram_tensor("be2", [D], F32, kind="ExternalInput")
    out_d = nc.dram_tensor("out", [b_loc, t_len, D], F32, kind="ExternalOutput")

    n_tt = t_len // 128                     # token tiles
    IC = 512 if t_len % 512 == 0 else t_len
    n_ic = t_len // IC
    NMA = min(512, t_len)                   # stage-A projection chunk

    with tile.TileContext(nc) as tc, ExitStack() as ctx:
        consts = ctx.enter_context(tc.tile_pool(name="consts", bufs=1))

        iden = consts.tile([128, 128], F32)
        make_identity(nc, iden)

        eps128 = consts.tile([128, 1], F32)
        nc.vector.memset(eps128, LN_EPS)

        # --- q/k projection weights: [40, 128] f32r, col 32h+e = W[h, :, e]
        def load_wqk(w_dram, name):
            t_ = consts.tile([D, 128], F32, tag=name)
            nc.vector.memset(t_, 0.0)
            for h in range(H):
                nc.sync.dma_start(out=t_[:, 32 * h : 32 * h + HS],
                                  in_=w_dram[h])
            t_r = consts.tile([D, 128], F32R, tag=name + "r")
            nc.vector.tensor_copy(out=t_r, in_=t_)
            return t_r

        wqT_r = load_wqk(wq_d, "wqT")
        wkT_r = load_wqk(wk_d, "wkT")

        ones_row_t = consts.tile([1, 128], F32)
        nc.vector.memset(ones_row_t, 1.0)
        ones_row_w = consts.tile([1, t_len], F32)
        nc.vector.memset(ones_row_w, 1.0)

        # --- v weights [41, 128]: col 32h+e = Wv[h,:,e]; the extra
        #     row 40 has 1.0 at cols 32h+10 (ones feature -> softmax Z)
        wvf = consts.tile([D + 1, 128], F32)
        nc.vector.memset(wvf, 0.0)
        for h in range(H):
            nc.sync.dma_start(out=wvf[0:D, 32 * h : 32 * h + HS], in_=wv_d[h])
            nc.sync.dma_start(
                out=wvf[D : D + 1, 32 * h + HS : 32 * h + HS + 1],
                in_=ones_row_t[0:1, 0:1])
        wv128e = consts.tile([D + 1, 128], F32R)
        nc.vector.tensor_copy(out=wv128e, in_=wvf)

        # Wp packed [128, 40]: row 32h+e = Wp[10h+e, :]; other rows zero
        wpp = consts.tile([128, D], F32)
        nc.vector.memset(wpp, 0.0)
        for h in range(H):
            nc.sync.dma_start(
                out=wpp[32 * h : 32 * h + HS, :],
                in_=wp_d[HS * h : HS * h + HS, :],
            )
        wpp_r = consts.tile([128, D], F32R)
        nc.vector.tensor_copy(out=wpp_r, in_=wpp)

        w1s = consts.tile([D, D], F32)
        nc.sync.dma_start(out=w1s, in_=w1_d[:])
        w1r = consts.tile([D, D], F32R)
        nc.vector.tensor_copy(out=w1r, in_=w1s)
        w2s = consts.tile([D, D], F32)
        nc.sync.dma_start(out=w2s, in_=w2_d[:])
        w2r = consts.tile([D, D], F32R)
        nc.vector.tensor_copy(out=w2r, in_=w2s)

        def load_col(v_dram, name):
            t_ = consts.tile([D, 1], F32, tag=name)
            nc.sync.dma_start(out=t_, in_=v_dram[:].unsqueeze(1))
            return t_

        bpc = load_col(bp_d, "bpc")
        b1c = load_col(b1_d, "b1c")
        b2c = load_col(b2_d, "b2c")
        g1c = load_col(g1_d, "g1c")
        be1c = load_col(be1_d, "be1c")
        g2c = load_col(g2_d, "g2c")
        be2c = load_col(be2_d, "be2c")

        # sel [128,128]: row 32h+10 has ones in cols 32h..32h+31 -> Z broadcast
        sel = consts.tile([128, 128], F32)
        nc.gpsimd.memset(sel, 0.0)
        nc.gpsimd.affine_select(
            out=sel[:].rearrange("p (g w) -> p g w", w=32),
            in_=sel[:].rearrange("p (g w) -> p g w", w=32),
            compare_op=OP.not_equal,
            fill=1.0,
            base=-HS,
            pattern=[[-32, 4], [0, 32]],
            channel_multiplier=1,
        )
        sel_r = consts.tile([128, 128], F32R)
        nc.vector.tensor_copy(out=sel_r, in_=sel)

        # mr_sel [2, 104]: row0 = 1 at cols 0:40, row1 = 1 at cols 64:104
        # (rstd broadcast lands at partitions 64:104 -- 32-aligned reads)
        MR2 = 64 + D
        mr_sel = consts.tile([2, MR2], F32)
        nc.vector.memset(mr_sel, 0.0)
        nc.vector.memset(mr_sel[0:1, 0:D], 1.0)
        nc.sync.dma_start(out=mr_sel[1:2, 64:MR2],
                          in_=ones_row_t[0:1, 0:D])
        mr_sel_r = consts.tile([2, MR2], F32R)
        nc.vector.tensor_copy(out=mr_sel_r, in_=mr_sel)

        # ones/D column [40,1] for LN2 mean matmuls
        onesD_f = consts.tile([D, 1], F32)
        nc.vector.memset(onesD_f, 1.0 / D)
        onesD = consts.tile([D, 1], F32R)
        nc.vector.tensor_copy(out=onesD, in_=onesD_f)

        # ---------------- per-batch persistent SBUF ----------------
        persist = ctx.enter_context(tc.tile_pool(name="persist", bufs=1))
        xnT = [persist.tile([D + 1, t_len], F32R, tag=f"xnT{b}", name=f"xnT{b}")
               for b in range(b_loc)]
        qT = [persist.tile([128, t_len], BF16, tag=f"qT{b}", name=f"qT{b}")
              for b in range(b_loc)]
        kT = [persist.tile([128, t_len], BF16, tag=f"kT{b}", name=f"kT{b}")
              for b in range(b_loc)]
        vA = [persist.tile([128, n_tt, 128], BF16, tag=f"vA{b}", name=f"vA{b}")
              for b in range(b_loc)]

        # stage-A working pools
        sbA = ctx.enter_context(tc.tile_pool(name="sbA", bufs=4))
        xtp = ctx.enter_context(tc.tile_pool(name="xtp", bufs=n_tt + 1))

        with (
            tc.tile_pool(name="sab", bufs=2, space="PSUM") as sabp,
            tc.tile_pool(name="sc", bufs=1, space="PSUM") as scp,
            tc.tile_pool(name="sd", bufs=1, space="PSUM") as sdp,
            tc.tile_pool(name="pvp", bufs=1, space="PSUM") as pvp,
            tc.tile_pool(name="misc", bufs=1, space="PSUM") as mp,
            tc.tile_pool(name="eab", bufs=3) as eabp,
            tc.tile_pool(name="ecd", bufs=2) as ecdp,
            tc.tile_pool(name="sbC", bufs=2) as sC,
            tc.tile_pool(name="outp", bufs=4) as op_,
        ):
            # ================= stage A =================
            def emit_stage_a_slices(b, prologue):
                mv = persist.tile([128, n_tt, 2], F32, tag=f"mv{b}",
                                  name=f"mv{b}")
                rstd = persist.tile([128, n_tt], F32, tag=f"rstd{b}",
                                    name=f"rstd{b}")
                GRP = min(4, n_tt)
                slices = []
                if prologue:
                    # before attention starts, all attention PSUM banks are
                    # free: rotate across them for deep pipelining
                    rings = [(sabp, "ab"), (scp, "c"), (sdp, "d"), (mp, "m")]
                else:
                    rings = [(mp, "m")]
                rk = [0]

                def psum(shape, name):
                    pool, tag = rings[rk[0] % len(rings)]
                    rk[0] += 1
                    return pool.tile(shape, F32, tag=tag, name=name)

                def ones_row(b=b):
                    nc.sync.dma_start(out=xnT[b][D : D + 1, :],
                                      in_=ones_row_w[:].bitcast(F32R))

                slices.append(ones_row)

                def ln_group(g0, b=b, mv=mv, rstd=rstd):
                    xts = {}
                    for t_i in range(g0, g0 + GRP):
                        xt = xtp.tile([128, D], F32, tag="xt", name="xt")
                        nc.sync.dma_start(
                            out=xt, in_=x_d[b, t_i * 128 : (t_i + 1) * 128, :])
                        st6 = sbA.tile([128, 6], F32, tag="st6", name="st6")
                        nc.vector.bn_stats(out=st6, in_=xt)
                        nc.vector.bn_aggr(out=mv[:, t_i, :], in_=st6)
                        xts[t_i] = xt
                    lnv = sbA.tile([128, GRP], F32, tag="lnv", name="lnv")
                    nc.scalar.activation(out=lnv, in_=mv[:, g0 : g0 + GRP, 1],
                                         func=AF.Ln, bias=eps128, scale=1.0)
                    nc.scalar.activation(out=rstd[:, g0 : g0 + GRP], in_=lnv,
                                         func=AF.Exp, bias=0.0, scale=-0.5)
                    xn_eng = nc.vector if prologue else nc.gpsimd
                    for t_i in range(g0, g0 + GRP):
                        xn = sbA.tile([128, D], F32, tag="xn", name="xn")
                        xn_eng.tensor_scalar(
                            out=xn, in0=xts[t_i],
                            scalar1=mv[:, t_i, 0:1],
                            scalar2=rstd[:, t_i : t_i + 1],
                            op0=OP.subtract, op1=OP.mult)
                        tp = psum([D, 128], "tp")
                        nc.tensor.transpose(tp, xn, iden)
                        # fold ln1's gamma/beta into the PSUM evacuation
                        nc.vector.tensor_scalar(
                            out=xnT[b][0:D, t_i * 128 : (t_i + 1) * 128],
                            in0=tp, scalar1=g1c, scalar2=be1c,
                            op0=OP.mult, op1=OP.add)

                def qk_chunk(c, b=b):
                    sl = slice(c * NMA, (c + 1) * NMA)
                    qp = psum([128, NMA], "qp")
                    nc.tensor.matmul(qp, lhsT=wqT_r, rhs=xnT[b][0:D, sl],
                                     start=True, stop=True)
                    nc.vector.tensor_copy(out=qT[b][:, sl], in_=qp)
                    kp = psum([128, NMA], "kp")
                    nc.tensor.matmul(kp, lhsT=wkT_r, rhs=xnT[b][0:D, sl],
                                     start=True, stop=True)
                    nc.vector.tensor_copy(out=kT[b][:, sl], in_=kp)

                def v_pair(g0, b=b):
                    for t_i in range(g0, min(g0 + 2, n_tt)):
                        vp = psum([128, 128], "vp")
                        nc.tensor.matmul(
                            vp,
                            lhsT=xnT[b][:, t_i * 128 : (t_i + 1) * 128],
                            rhs=wv128e, start=True, stop=True)
                        nc.vector.tensor_copy(out=vA[b][:, t_i, :], in_=vp)

                for g0 in range(0, n_tt, GRP):
                    slices.append(lambda g0=g0: ln_group(g0))
                    slices.append(lambda c=g0 // GRP: qk_chunk(c))
                    slices.append(lambda g0=g0: v_pair(g0))
                    slices.append(lambda g0=g0 + 2: v_pair(g0))
                return slices, rings

            # ================= stage C =================
            def make_stage_c_pieces(b, i0, hold):
                st = {}
                gsl = slice(i0, i0 + IC)

                def p1():
                    pv_sb = hold["sb"]
                    zbc = mp.tile([128, IC], F32, tag="m", name="zbc")
                    nc.tensor.matmul(zbc, lhsT=sel_r, rhs=pv_sb,
                                     start=True, stop=True)
                    # 1/Z = exp(-ln(Z)) on ACT (reads PSUM directly);
                    # keeps the expensive reciprocal off the DVE
                    lnz = sC.tile([128, IC], F32, tag="lnz", name="lnz")
                    nc.scalar.activation(out=lnz, in_=zbc, func=AF.Ln,
                                         bias=0.0, scale=1.0)
                    rbc = sC.tile([128, IC], F32, tag="rbc", name="rbc")
                    nc.scalar.activation(out=rbc, in_=lnz, func=AF.Exp,
                                         bias=0.0, scale=-1.0)
                    st["rbc"] = rbc

                def p2():
                    on = sC.tile([128, IC], F32R, tag="on", name="on")
                    nc.gpsimd.tensor_mul(out=on, in0=hold["sb"], in1=st["rbc"])
                    st["on"] = on

                def p3():
                    yp = mp.tile([D, IC], F32, tag="m", name="yp")
                    nc.tensor.matmul(yp, lhsT=wpp_r, rhs=st["on"],
                                     start=True, stop=True)
                    x1 = sC.tile([D, IC], F32R, tag="x1", name="x1")
                    nc.vector.scalar_tensor_tensor(
                        out=x1, in0=yp, scalar=bpc, in1=xnT[b][0:D, gsl],
                        op0=OP.add, op1=OP.add)
                    st["x1"] = x1

                def p4():
                    sq = sC.tile([D, IC], F32R, tag="sq", name="sq")
                    nc.gpsimd.tensor_mul(out=sq, in0=st["x1"], in1=st["x1"])
                    st["sq"] = sq
                    mup = mp.tile([1, IC], F32, tag="m", name="mup")
                    nc.tensor.matmul(mup, lhsT=onesD, rhs=st["x1"],
                                     start=True, stop=True)
                    mus = sC.tile([1, IC], F32R, tag="mus", name="mus")
                    nc.scalar.copy(out=mus, in_=mup)
                    mr = sC.tile([2, IC], F32R, tag="mr", name="mr")
                    nc.sync.dma_start(out=mr[0:1, :], in_=mus[:])
                    st["mus"] = mus
                    st["mr"] = mr

                def p5():
                    m2p = mp.tile([1, IC], F32, tag="m", name="m2p")
                    nc.tensor.matmul(m2p, lhsT=onesD, rhs=st["sq"],
                                     start=True, stop=True)
                    msq = sC.tile([1, IC], F32, tag="msq", name="msq")
                    nc.gpsimd.tensor_mul(out=msq, in0=st["mus"],
                                         in1=st["mus"])
                    var = sC.tile([1, IC], F32, tag="var", name="var")
                    nc.vector.tensor_sub(out=var, in0=m2p, in1=msq)
                    lnv2 = sC.tile([1, IC], F32, tag="lnv2", name="lnv2")
                    nc.scalar.activation(out=lnv2, in_=var, func=AF.Ln,
                                         bias=eps128[0:1, :], scale=1.0)
                    rsd = sC.tile([1, IC], F32R, tag="rsd", name="rsd")
                    nc.scalar.activation(out=rsd, in_=lnv2,
                                         func=AF.Exp, bias=0.0, scale=-0.5)
                    nc.sync.dma_start(out=st["mr"][1:2, :], in_=rsd[:])

                def p6():
                    mrbc = mp.tile([MR2, IC], F32, tag="m", name="mrbc")
                    nc.tensor.matmul(mrbc, lhsT=mr_sel_r, rhs=st["mr"],
                                     start=True, stop=True)
                    t1 = sC.tile([D, IC], F32, tag="t1", name="t1")
                    nc.vector.tensor_sub(out=t1, in0=st["x1"], in1=mrbc[0:D, :])
                    t2 = sC.tile([D, IC], F32R, tag="t2", name="t2")
                    nc.vector.tensor_mul(out=t2, in0=t1, in1=mrbc[64:MR2, :])
                    x2 = sC.tile([D, IC], F32R, tag="x2", name="x2")
                    nc.gpsimd.tensor_scalar(out=x2, in0=t2, scalar1=g2c,
                                            scalar2=be2c, op0=OP.mult,
                                            op1=OP.add)
                    st["x2"] = x2

                def p7():
                    hp = mp.tile([D, IC], F32, tag="m", name="hp")
                    nc.tensor.matmul(hp, lhsT=w1r, rhs=st["x2"],
                                     start=True, stop=True)
                    hs = sC.tile([D, IC], F32R, tag="hs", name="hs")
                    nc.vector.tensor_scalar(
                        out=hs, in0=hp, scalar1=b1c, scalar2=0.0,
                        op0=OP.add, op1=OP.max)
                    st["hs"] = hs

                def p8():
                    y2p = mp.tile([D, IC], F32, tag="m", name="y2p")
                    nc.tensor.matmul(y2p, lhsT=w2r, rhs=st["hs"],
                                     start=True, stop=True)
                    ob = sC.tile([D, IC], F32, tag="ob", name="ob")
                    nc.vector.scalar_tensor_tensor(
                        out=ob, in0=y2p, scalar=b2c, in1=st["x2"],
                        op0=OP.add, op1=OP.add)
                    st["ob"] = ob

                def out_piece(tt0):
                    for tt_i in range(tt0, min(tt0 + 2, IC // 128)):
                        otp = mp.tile([128, D], F32, tag="m", name="otp")
                        nc.tensor.transpose(
                            otp, st["ob"][:, tt_i * 128 : (tt_i + 1) * 128],
                            iden[0:D, 0:D])
                        osb = op_.tile([128, D], F32, tag="osb", name="osb")
                        nc.vector.tensor_copy(out=osb, in_=otp)
                        t_glob = i0 + tt_i * 128
                        nc.sync.dma_start(
                            out=out_d[b, t_glob : t_glob + 128, :], in_=osb)

                return [p1, p2, p3, p4, p5, p6, p7, p8,
                        lambda: out_piece(0), lambda: out_piece(2)]

            # ================= attention main loop =================
            # emit only the first 512 tokens' worth of batch-0 stage A up
            # front (what chunk 0 needs to start); drip the rest into the
            # chunk-0 attention stream
            a0_slices, a0_rings = emit_stage_a_slices(0, prologue=True)
            for f in a0_slices[:5]:
                f()
            a0_rings[:] = [(mp, "m")]
            a_queue = list(a0_slices[5:])
            for b2 in range(1, b_loc):
                s2, _ = emit_stage_a_slices(b2, prologue=False)
                a_queue.extend(s2)

            c_queue = []
            prev_pv = [None]

            def emit_pv(b, j, eab, ec, ed_src, pv, hold):
                srcs = [eab[:, 0, :], eab[:, 1, :],
                        ec[:].bitcast(BF16), ed_src]
                for h in range(H):
                    nc.tensor.matmul(
                        pv[32 * h : 32 * h + 32, :],
                        lhsT=vA[b][:, j, 32 * h : 32 * h + 32],
                        rhs=srcs[h],
                        start=(j == 0), stop=(j == n_tt - 1),
                        skip_group_check=True,
                        tile_position=(0, 32 * h))
                if j == n_tt - 1:
                    pv_sb = sC.tile([128, IC], F32R, tag="pvsb", name="pv_sb")
                    nc.scalar.copy(out=pv_sb, in_=pv)
                    hold["sb"] = pv_sb

            gstep = [0]
            for b in range(b_loc):
                if b > 0:
                    while a_queue:
                        a_queue.pop(0)()
                for ic in range(n_ic):
                    i0 = ic * IC
                    isl = slice(i0, i0 + IC)
                    pv = pvp.tile([128, IC], F32, tag="pv")
                    hold = {}
                    for j in range(n_tt):
                        jsl = slice(j * 128, (j + 1) * 128)
                        # S matmuls: heads 2,3 first (their PSUM banks have
                        # the tightest turnaround), then 0,1; all adjacent.
                        s_c = scp.tile([128, IC], F32, tag="c", name="s_c")
                        s_d = sdp.tile([128, IC], F32, tag="d", name="s_d")
                        s_ab = sabp.tile([128, 2, IC], F32, tag="ab",
                                         name="s_ab")
                        for h, dst in ((2, s_c[:, 0:IC]), (3, s_d[:, 0:IC]),
                                       (0, s_ab[:, 0, 0:IC]),
                                       (1, s_ab[:, 1, 0:IC])):
                            hp = slice(32 * h, 32 * h + HS)
                            nc.tensor.matmul(
                                dst,
                                lhsT=qT[b][hp, jsl],
                                rhs=kT[b][hp, isl],
                                start=True, stop=True,
                                tile_position=(32 * h, 0))
                        # exps: DVE fast-exp head 2 (+3 odd j); ACT exact
                        # for 0,1 (+3 even j -- load balance ACT vs DVE)
                        ec = ecdp.tile([128, IC], I16, tag="ec", name="ec")
                        nc.vector.tensor_scalar(
                            out=ec, in0=s_c, scalar1=SCH_A, scalar2=SCH_B,
                            op0=OP.mult, op1=OP.add)
                        if j % 2 == 0:
                            ed = ecdp.tile([128, IC], BF16, tag="eda",
                                           name="eda")
                            nc.scalar.activation(out=ed, in_=s_d, func=AF.Exp)
                            ed_src = ed[:]
                        else:
                            ed = ecdp.tile([128, IC], I16, tag="ed", name="ed")
                            nc.vector.tensor_scalar(
                                out=ed, in0=s_d, scalar1=SCH_A, scalar2=SCH_B,
                                op0=OP.mult, op1=OP.add)
                            ed_src = ed[:].bitcast(BF16)
                        eab = eabp.tile([128, 2, IC], BF16, tag="eab",
                                        name="eab")
                        nc.scalar.activation(out=eab[:, :, 0:IC],
                                             in_=s_ab[:, :, 0:IC],
                                             func=AF.Exp)
                        # PV for previous j (its e tiles are long since done)
                        if prev_pv[0] is not None:
                            prev_pv[0]()
                        prev_pv[0] = (
                            lambda b=b, j=j, eab=eab, ec=ec, ed_src=ed_src,
                                   pv=pv, hold=hold:
                            emit_pv(b, j, eab, ec, ed_src, pv, hold))
                        # drip deferred work into the stream
                        if c_queue:
                            c_queue.pop(0)()
                        elif a_queue:
                            a_queue.pop(0)()
                        gstep[0] += 1
                    c_queue.extend(make_stage_c_pieces(b, i0, hold))
            prev_pv[0]()
            while c_queue:
                c_queue.pop(0)()

    if split_waits:
        _split_multiwaits(nc)
    return nc


def _split_multiwaits(nc):
    """walrus codegen in this container encodes a limited number of sem
    waits per instruction (1 for Drain, 2 for compute ops); spill extras
    onto preceding NOPs on the same engine. DMA copies are left alone —
    their waits ride in the DGE descriptor."""
    for func in nc.m.functions:
        for bb in func.blocks:
            insts = list(bb.instructions)
            out, changed = [], False
            for ins in insts:
                si = ins.sync_info
                maxw = 1
                if (maxw is not None and si is not None and si.on_wait
                        and len(si.on_wait) > maxw):
                    waits = list(si.on_wait)
                    for k, w in enumerate(waits[:-maxw]):
                        nop = mybir.InstNoOp(
                            name=f"{ins.name}-wsplit{k}",
                            sync_info=mybir.SyncInfo(on_wait=[w], on_update=[]),
                            bass_nofuse=True, engine=ins.engine)
                        try:
                            nc.register_instruction(nop, overwrite=True)
                        except Exception:
                            pass
                        out.append(nop)
                    si.on_wait = waits[-maxw:]
                    changed = True
                out.append(ins)
            if changed:
                bb.instructions = out


_NC_CACHE = {}


def kernel(**inputs):
    from concourse.bass_utils import run_bass_kernel_spmd

    x = np.ascontiguousarray(np.asarray(inputs["x"], dtype=np.float32))
    b_full = x.shape[0]
    n_cores = N_CORES
    b_loc = b_full // n_cores

    key = (b_loc, x.shape[1])
    if key not in _NC_CACHE:
        _NC_CACHE[key] = build_kernel(b_loc, x.shape[1])
    nc = _NC_CACHE[key]

    weights = {k: np.ascontiguousarray(np.asarray(inputs[k], dtype=np.float32))
               for k in ("Wq", "Wk", "Wv", "Wp", "bp", "W1", "b1", "W2", "b2",
                         "g1", "be1", "g2", "be2")}
    in_maps = []
    for c in range(n_cores):
        m = {"x": x[c * b_loc : (c + 1) * b_loc]}
        m.update(weights)
        in_maps.append(m)

    res = run_bass_kernel_spmd(nc, in_maps, core_ids=list(range(n_cores)))
    out = np.concatenate([r["out"] for r in res.results], axis=0)
    return out


# revision 37
# speedup vs baseline: 1.0340x; 1.0340x over previous
"""Trainium2 Bass kernel for a small dense transformer block (v2).

Model (per reference):
  x   : [B, T, D]  B=16, T=2048, D=40, H=4 heads, hs=10
  ln1 -> per-head q/k/v -> scores = k @ q^T (softmax over q index) -> out @ Wp
  residual (on ln1(x)) -> ln2 -> FFN(relu) -> residual (on ln2 output)

Sharding: data-parallel over batch, 2 batches per core across 8 cores.

v2 design notes (vs v1):
  - q and k packed in ONE bf16 buffer qkT [128, T]: head h has q at
    partitions 32h..32h+9 and k at partitions 32h+16..32h+25. One stage-A
    matmul per 512 tokens produces all of q+k for all heads; S matmuls
    become bf16 (1 cyc/row vs ~1.5-4 for fp32/f32r).
  - 4 S matmuls per j-tile emitted adjacently to 4 distinct PSUM banks
    (row-group tiling) so they can overlap in the PE.
  - exp split across engines: heads 0,1 via exact ACT exp (bf16 out),
    heads 2,3 via a Schraudolph-style fast exp on the DVE: one
    tensor_scalar producing int16 bf16-bit-patterns, bitcast to bf16 for
    the PV matmul. End-to-end rel err ~3e-3 (gate 2e-2).
  - v has an extra "ones" feature via a ones row in xnT (partition 40)
    and a ones entry in the packed v weights, giving the softmax
    denominator for free in the PV accumulation (as v1, minus the
    per-tile ones writes).
  - LN2 mean/rstd broadcast fused into ONE matmul ([2,512] f32r moving,
    [2,80] one-hot stationary -> [80,512]: rows 0:40 mu, 40:80 rstd).
  - reciprocal -> reciprocal_approx_fast (5x).
  - elementwise ops distributed: gpsimd takes SBUF-only ops (on, sq, x2,
    xn, msq), ACT takes PSUM-reading copies/activations it has slack for
    (pv_sb, mus, lnv2, rsd, hs-relu), DVE keeps the rest.
"""

import sys
from contextlib import ExitStack

for _p in ("/opt/trn_rl_repo",):
    if _p not in sys.path:
        sys.path.insert(0, _p)

import numpy as np

import concourse.bass as bass
import concourse.tile as tile
from concourse import mybir
from concourse.masks import make_identity

B_FULL = 16
N_CORES = 8
B_LOC = B_FULL // N_CORES
T = 2048
D = 40
H = 4
HS = 10
LN_EPS = 1e-5

F32 = mybir.dt.float32
F32R = mybir.dt.float32r
BF16 = mybir.dt.bfloat16
I16 = mybir.dt.int16
AF = mybir.ActivationFunctionType
OP = mybir.AluOpType

# Schraudolph fast-exp constants for bf16 bit patterns via int16:
#   bits = round(s * 128/ln2 + (127*128 - C))
SCH_A = 128.0 / float(np.log(2.0))
SCH_B = 127.0 * 128.0 - 5.5


def build_kernel(b_loc=B_LOC, t_len=T, split_waits=True):
    nc = bass.Bass("TRN2", target_bir_lowering=False)

    x_d = nc.dram_tensor("x", [b_loc, t_len, D], F32, kind="ExternalInput")
    wq_d = nc.dram_tensor("Wq", [H, D, HS], F32, kind="ExternalInput")
    wk_d = nc.dram_tensor("Wk", [H, D, HS], F32, kind="ExternalInput")
    wv_d = nc.dram_tensor("Wv", [H, D, HS], F32, kind="ExternalInput")
    wp_d = nc.dram_tensor("Wp", [D, D], F32, kind="ExternalInput")
    bp_d = nc.dram_tensor("bp", [D], F32, kind="ExternalInput")
    w1_d = nc.dram_tensor("W1", [D, D], F32, kind="ExternalInput")
    b1_d = nc.dram_tensor("b1", [D], F32, kind="ExternalInput")
    w2_d = nc.dram_tensor("W2", [D, D], F32, kind="ExternalInput")
    b2_d = nc.dram_tensor("b2", [D], F32, kind="ExternalInput")
    g1_d = nc.dram_tensor("g1", [D], F32, kind="ExternalInput")
    be1_d = nc.dram_tensor("be1", [D], F32, kind="ExternalInput")
    g2_d = nc.dram_tensor("g2", [D], F32, kind="ExternalInput")
    be2_d = nc.dram_tensor("be2", [D], F32, kind="ExternalInput")
    out_d = nc.dram_tensor("out", [b_loc, t_len, D], F32, kind="ExternalOutput")

    n_tt = t_len // 128                     # token tiles
    IC = 512 if t_len % 512 == 0 else t_len
    n_ic = t_len // IC
    NMA = min(512, t_len)                   # stage-A projection chunk

    with tile.TileContext(nc) as tc, ExitStack() as ctx:
        consts = ctx.enter_context(tc.tile_pool(name="consts", bufs=1))

        iden = consts.tile([128, 128], F32)
        make_identity(nc, iden)

        eps128 = consts.tile([128, 1], F32)
        nc.vector.memset(eps128, LN_EPS)

        # --- q/k projection weights: [40, 128] f32r, col 32h+e = W[h, :, e]
        def load_wqk(w_dram, name):
            t_ = consts.tile([D, 128], F32, tag=name)
            nc.vector.memset(t_, 0.0)
            for h in range(H):
                nc.sync.dma_start(out=t_[:, 32 * h : 32 * h + HS],
                                  in_=w_dram[h])
            t_r = consts.tile([D, 128], F32R, tag=name + "r")
            nc.vector.tensor_copy(out=t_r, in_=t_)
            return t_r

        wqT_r = load_wqk(wq_d, "wqT")
        wkT_r = load_wqk(wk_d, "wkT")

        ones_row_t = consts.tile([1, 128], F32)
        nc.vector.memset(ones_row_t, 1.0)
        ones_row_w = consts.tile([1, t_len], F32)
        nc.vector.memset(ones_row_w, 1.0)

        # --- v weights [41, 128]: col 32h+e = Wv[h,:,e]; the extra
        #     row 40 has 1.0 at cols 32h+10 (ones feature -> softmax Z)
        wvf = consts.tile([D + 1, 128], F32)
        nc.vector.memset(wvf, 0.0)
        for h in range(H):
            nc.sync.dma_start(out=wvf[0:D, 32 * h : 32 * h + HS], in_=wv_d[h])
            nc.sync.dma_start(
                out=wvf[D : D + 1, 32 * h + HS : 32 * h + HS + 1],
                in_=ones_row_t[0:1, 0:1])
        wv128e = consts.tile([D + 1, 128], F32R)
        nc.vector.tensor_copy(out=wv128e, in_=wvf)

        # Wp packed [128, 40]: row 32h+e = Wp[10h+e, :]; other rows zero
        wpp = consts.tile([128, D], F32)
        nc.vector.memset(wpp, 0.0)
        for h in range(H):
            nc.sync.dma_start(
                out=wpp[32 * h : 32 * h + HS, :],
                in_=wp_d[HS * h : HS * h + HS, :],
            )
        wpp_r = consts.tile([128, D], F32R)
        nc.vector.tensor_copy(out=wpp_r, in_=wpp)

        w1s = consts.tile([D, D], F32)
        nc.sync.dma_start(out=w1s, in_=w1_d[:])
        w1r = consts.tile([D, D], F32R)
        nc.vector.tensor_copy(out=w1r, in_=w1s)
        w2s = consts.tile([D, D], F32)
        nc.sync.dma_start(out=w2s, in_=w2_d[:])
        w2r = consts.tile([D, D], F32R)
        nc.vector.tensor_copy(out=w2r, in_=w2s)

        def load_col(v_dram, name):
            t_ = consts.tile([D, 1], F32, tag=name)
            nc.sync.dma_start(out=t_, in_=v_dram[:].unsqueeze(1))
            return t_

        bpc = load_col(bp_d, "bpc")
        b1c = load_col(b1_d, "b1c")
        b2c = load_col(b2_d, "b2c")
        g1c = load_col(g1_d, "g1c")
        be1c = load_col(be1_d, "be1c")
        g2c = load_col(g2_d, "g2c")
        be2c = load_col(be2_d, "be2c")

        # sel [128,128]: row 32h+10 has ones in cols 32h..32h+31 -> Z broadcast
        sel = consts.tile([128, 128], F32)
        nc.gpsimd.memset(sel, 0.0)
        nc.gpsimd.affine_select(
            out=sel[:].rearrange("p (g w) -> p g w", w=32),
            in_=sel[:].rearrange("p (g w) -> p g w", w=32),
            compare_op=OP.not_equal,
            fill=1.0,
            base=-HS,
            pattern=[[-32, 4], [0, 32]],
            channel_multiplier=1,
        )
        sel_r = consts.tile([128, 128], F32R)
        nc.vector.tensor_copy(out=sel_r, in_=sel)

        # mr_sel [2, 104]: row0 = 1 at cols 0:40, row1 = 1 at cols 64:104
        # (rstd broadcast lands at partitions 64:104 -- 32-aligned reads)
        MR2 = 64 + D
        mr_sel = consts.tile([2, MR2], F32)
        nc.vector.memset(mr_sel, 0.0)
        nc.vector.memset(mr_sel[0:1, 0:D], 1.0)
        nc.sync.dma_start(out=mr_sel[1:2, 64:MR2],
                          in_=ones_row_t[0:1, 0:D])
        mr_sel_r = consts.tile([2, MR2], F32R)
        nc.vector.tensor_copy(out=mr_sel_r, in_=mr_sel)

        # ones/D column [40,1] for LN2 mean matmuls
        onesD_f = consts.tile([D, 1], F32)
        nc.vector.memset(onesD_f, 1.0 / D)
        onesD = consts.tile([D, 1], F32R)
        nc.vector.tensor_copy(out=onesD, in_=onesD_f)

        # ---------------- per-batch persistent SBUF ----------------
        persist = ctx.enter_context(tc.tile_pool(name="persist", bufs=1))
        xnT = [persist.tile([D + 1, t_len], F32R, tag=f"xnT{b}", name=f"xnT{b}")
               for b in range(b_loc)]
        qT = [persist.tile([128, t_len], BF16, tag=f"qT{b}", name=f"qT{b}")
              for b in range(b_loc)]
        kT = [persist.tile([128, t_len], BF16, tag=f"kT{b}", name=f"kT{b}")
              for b in range(b_loc)]
        vA = [persist.tile([128, n_tt, 128], BF16, tag=f"vA{b}", name=f"vA{b}")
              for b in range(b_loc)]

        # stage-A working pools
        sbA = ctx.enter_context(tc.tile_pool(name="sbA", bufs=4))
        xtp = ctx.enter_context(tc.tile_pool(name="xtp", bufs=n_tt + 1))

        with (
            tc.tile_pool(name="sab", bufs=2, space="PSUM") as sabp,
            tc.tile_pool(name="sc", bufs=1, space="PSUM") as scp,
            tc.tile_pool(name="sd", bufs=1, space="PSUM") as sdp,
            tc.tile_pool(name="pvp", bufs=1, space="PSUM") as pvp,
            tc.tile_pool(name="misc", bufs=1, space="PSUM") as mp,
            tc.tile_pool(name="eab", bufs=3) as eabp,
            tc.tile_pool(name="ecd", bufs=2) as ecdp,
            tc.tile_pool(name="sbC", bufs=2) as sC,
            tc.tile_pool(name="outp", bufs=4) as op_,
        ):
            # ================= stage A =================
            def emit_stage_a_slices(b, prologue):
                mv = persist.tile([128, n_tt, 2], F32, tag=f"mv{b}",
                                  name=f"mv{b}")
                rstd = persist.tile([128, n_tt], F32, tag=f"rstd{b}",
                                    name=f"rstd{b}")
                GRP = min(4, n_tt)
                slices = []
                if prologue:
                    # before attention starts, all attention PSUM banks are
                    # free: rotate across them for deep pipelining
                    rings = [(sabp, "ab"), (scp, "c"), (sdp, "d"), (mp, "m")]
                else:
                    rings = [(mp, "m")]
                rk = [0]

                def psum(shape, name):
                    pool, tag = rings[rk[0] % len(rings)]
                    rk[0] += 1
                    return pool.tile(shape, F32, tag=tag, name=name)

                def ones_row(b=b):
                    nc.sync.dma_start(out=xnT[b][D : D + 1, :],
                                      in_=ones_row_w[:].bitcast(F32R))

                slices.append(ones_row)

                def ln_group(g0, b=b, mv=mv, rstd=rstd):
                    xts = {}
                    for t_i in range(g0, g0 + GRP):
                        xt = xtp.tile([128, D], F32, tag="xt", name="xt")
                        # x loads ride the scalar engine's DGE queue so they
                        # don't serialize behind the weight loads on sync
                        nc.scalar.dma_start(
                            out=xt, in_=x_d[b, t_i * 128 : (t_i + 1) * 128, :])
                        st6 = sbA.tile([128, 6], F32, tag="st6", name="st6")
                        nc.vector.bn_stats(out=st6, in_=xt)
                        nc.vector.bn_aggr(out=mv[:, t_i, :], in_=st6)
                        xts[t_i] = xt
                    lnv = sbA.tile([128, GRP], F32, tag="lnv", name="lnv")
                    nc.scalar.activation(out=lnv, in_=mv[:, g0 : g0 + GRP, 1],
                                         func=AF.Ln, bias=eps128, scale=1.0)
                    nc.scalar.activation(out=rstd[:, g0 : g0 + GRP], in_=lnv,
                                         func=AF.Exp, bias=0.0, scale=-0.5)
                    xn_eng = nc.vector if prologue else nc.gpsimd
                    for t_i in range(g0, g0 + GRP):
                        xn = sbA.tile([128, D], F32, tag="xn", name="xn")
                        xn_eng.tensor_scalar(
                            out=xn, in0=xts[t_i],
                            scalar1=mv[:, t_i, 0:1],
                            scalar2=rstd[:, t_i : t_i + 1],
                            op0=OP.subtract, op1=OP.mult)
                        tp = psum([D, 128], "tp")
                        nc.tensor.transpose(tp, xn, iden)
                        # fold ln1's gamma/beta into the PSUM evacuation
                        nc.vector.tensor_scalar(
                            out=xnT[b][0:D, t_i * 128 : (t_i + 1) * 128],
                            in0=tp, scalar1=g1c, scalar2=be1c,
                            op0=OP.mult, op1=OP.add)

                def qk_chunk(c, b=b):
                    sl = slice(c * NMA, (c + 1) * NMA)
                    qp = psum([128, NMA], "qp")
                    nc.tensor.matmul(qp, lhsT=wqT_r, rhs=xnT[b][0:D, sl],
                                     start=True, stop=True)
                    nc.vector.tensor_copy(out=qT[b][:, sl], in_=qp)
                    kp = psum([128, NMA], "kp")
                    nc.tensor.matmul(kp, lhsT=wkT_r, rhs=xnT[b][0:D, sl],
                                     start=True, stop=True)
                    nc.vector.tensor_copy(out=kT[b][:, sl], in_=kp)

                def v_pair(g0, b=b):
                    for t_i in range(g0, min(g0 + 2, n_tt)):
                        vp = psum([128, 128], "vp")
                        nc.tensor.matmul(
                            vp,
                            lhsT=xnT[b][:, t_i * 128 : (t_i + 1) * 128],
                            rhs=wv128e, start=True, stop=True)
                        nc.vector.tensor_copy(out=vA[b][:, t_i, :], in_=vp)

                for g0 in range(0, n_tt, GRP):
                    slices.append(lambda g0=g0: ln_group(g0))
                    slices.append(lambda c=g0 // GRP: qk_chunk(c))
                    slices.append(lambda g0=g0: v_pair(g0))
                    slices.append(lambda g0=g0 + 2: v_pair(g0))
                return slices, rings

            # ================= stage C =================
            def make_stage_c_pieces(b, i0, hold, rings=None):
                st = {}
                gsl = slice(i0, i0 + IC)
                if rings is None:
                    rings = [(mp, "m")]
                rk = [0]

                def cpsum(shape, name):
                    pool, tag = rings[rk[0] % len(rings)]
                    rk[0] += 1
                    return pool.tile(shape, F32, tag=tag, name=name)

                def p1():
                    pv_sb = hold["sb"]
                    zbc = cpsum([128, IC], "zbc")
                    nc.tensor.matmul(zbc, lhsT=sel_r, rhs=pv_sb,
                                     start=True, stop=True)
                    # 1/Z = exp(-ln(Z)) on ACT (reads PSUM directly);
                    # keeps the expensive reciprocal off the DVE
                    lnz = sC.tile([128, IC], F32, tag="lnz", name="lnz")
                    nc.scalar.activation(out=lnz, in_=zbc, func=AF.Ln,
                                         bias=0.0, scale=1.0)
                    rbc = sC.tile([128, IC], F32, tag="rbc", name="rbc")
                    nc.scalar.activation(out=rbc, in_=lnz, func=AF.Exp,
                                         bias=0.0, scale=-1.0)
                    st["rbc"] = rbc

                def p2():
                    on = sC.tile([128, IC], F32R, tag="on", name="on")
                    nc.gpsimd.tensor_mul(out=on, in0=hold["sb"], in1=st["rbc"])
                    st["on"] = on

                def p3():
                    yp = cpsum([D, IC], "yp")
                    nc.tensor.matmul(yp, lhsT=wpp_r, rhs=st["on"],
                                     start=True, stop=True)
                    x1 = sC.tile([D, IC], F32R, tag="x1", name="x1")
                    nc.vector.scalar_tensor_tensor(
                        out=x1, in0=yp, scalar=bpc, in1=xnT[b][0:D, gsl],
                        op0=OP.add, op1=OP.add)
                    st["x1"] = x1

                def p4():
                    sq = sC.tile([D, IC], F32R, tag="sq", name="sq")
                    nc.gpsimd.tensor_mul(out=sq, in0=st["x1"], in1=st["x1"])
                    st["sq"] = sq
                    mup = cpsum([1, IC], "mup")
                    nc.tensor.matmul(mup, lhsT=onesD, rhs=st["x1"],
                                     start=True, stop=True)
                    mus = sC.tile([1, IC], F32R, tag="mus", name="mus")
                    nc.scalar.copy(out=mus, in_=mup)
                    mr = sC.tile([2, IC], F32R, tag="mr", name="mr")
                    nc.gpsimd.dma_start(out=mr[0:1, :], in_=mus[:])
                    st["mus"] = mus
                    st["mr"] = mr

                def p5():
                    m2p = cpsum([1, IC], "m2p")
                    nc.tensor.matmul(m2p, lhsT=onesD, rhs=st["sq"],
                                     start=True, stop=True)
                    msq = sC.tile([1, IC], F32, tag="msq", name="msq")
                    nc.gpsimd.tensor_mul(out=msq, in0=st["mus"],
                                         in1=st["mus"])
                    var = sC.tile([1, IC], F32, tag="var", name="var")
                    nc.vector.tensor_sub(out=var, in0=m2p, in1=msq)
                    lnv2 = sC.tile([1, IC], F32, tag="lnv2", name="lnv2")
                    nc.scalar.activation(out=lnv2, in_=var, func=AF.Ln,
                                         bias=eps128[0:1, :], scale=1.0)
                    rsd = sC.tile([1, IC], F32R, tag="rsd", name="rsd")
                    nc.scalar.activation(out=rsd, in_=lnv2,
                                         func=AF.Exp, bias=0.0, scale=-0.5)
                    nc.gpsimd.dma_start(out=st["mr"][1:2, :], in_=rsd[:])

                def p6():
                    mrbc = cpsum([MR2, IC], "mrbc")
                    nc.tensor.matmul(mrbc, lhsT=mr_sel_r, rhs=st["mr"],
                                     start=True, stop=True)
                    t1 = sC.tile([D, IC], F32, tag="t1", name="t1")
                    nc.vector.tensor_sub(out=t1, in0=st["x1"], in1=mrbc[0:D, :])
                    t2 = sC.tile([D, IC], F32R, tag="t2", name="t2")
                    nc.vector.tensor_mul(out=t2, in0=t1, in1=mrbc[64:MR2, :])
                    x2 = sC.tile([D, IC], F32R, tag="x2", name="x2")
                    nc.gpsimd.tensor_scalar(out=x2, in0=t2, scalar1=g2c,
                                            scalar2=be2c, op0=OP.mult,
                                            op1=OP.add)
                    st["x2"] = x2

                def p7():
                    hp = cpsum([D, IC], "hp")
                    nc.tensor.matmul(hp, lhsT=w1r, rhs=st["x2"],
                                     start=True, stop=True)
                    hs = sC.tile([D, IC], F32R, tag="hs", name="hs")
                    nc.vector.tensor_scalar(
                        out=hs, in0=hp, scalar1=b1c, scalar2=0.0,
                        op0=OP.add, op1=OP.max)
                    st["hs"] = hs

                def p8():
                    y2p = cpsum([D, IC], "y2p")
                    nc.tensor.matmul(y2p, lhsT=w2r, rhs=st["hs"],
                                     start=True, stop=True)
                    ob = sC.tile([D, IC], F32, tag="ob", name="ob")
                    nc.vector.scalar_tensor_tensor(
                        out=ob, in0=y2p, scalar=b2c, in1=st["x2"],
                        op0=OP.add, op1=OP.add)
                    st["ob"] = ob

                def out_piece(tt0):
                    for tt_i in range(tt0, min(tt0 + 2, IC // 128)):
                        otp = cpsum([128, D], "otp")
                        nc.tensor.transpose(
                            otp, st["ob"][:, tt_i * 128 : (tt_i + 1) * 128],
                            iden[0:D, 0:D])
                        osb = op_.tile([128, D], F32, tag="osb", name="osb")
                        nc.vector.tensor_copy(out=osb, in_=otp)
                        t_glob = i0 + tt_i * 128
                        nc.sync.dma_start(
                            out=out_d[b, t_glob : t_glob + 128, :], in_=osb)

                return [p1, p2, p3, p4, p5, p6, p7, p8,
                        lambda: out_piece(0), lambda: out_piece(2)]

            # ================= attention main loop =================
            # emit only the first 512 tokens' worth of batch-0 stage A up
            # front (what chunk 0 needs to start); drip the rest into the
            # chunk-0 attention stream
            a0_slices, a0_rings = emit_stage_a_slices(0, prologue=True)
            for f in a0_slices[:5]:
                f()
            a0_rings[:] = [(mp, "m")]
            a_queue = list(a0_slices[5:])
            for b2 in range(1, b_loc):
                s2, _ = emit_stage_a_slices(b2, prologue=False)
                a_queue.extend(s2)

            c_queue = []
            prev_pv = [None]

            def emit_pv(b, j, eab, ec, ed_src, pv, hold):
                srcs = [eab[:, 0, :], eab[:, 1, :],
                        ec[:].bitcast(BF16), ed_src]
                for h in range(H):
                    nc.tensor.matmul(
                        pv[32 * h : 32 * h + 32, :],
                        lhsT=vA[b][:, j, 32 * h : 32 * h + 32],
                        rhs=srcs[h],
                        start=(j == 0), stop=(j == n_tt - 1),
                        skip_group_check=True,
                        tile_position=(0, 32 * h))
                if j == n_tt - 1:
                    pv_sb = sC.tile([128, IC], F32R, tag="pvsb", name="pv_sb")
                    nc.scalar.copy(out=pv_sb, in_=pv)
                    hold["sb"] = pv_sb

            gstep = [0]
            for b in range(b_loc):
                if b > 0:
                    while a_queue:
                        a_queue.pop(0)()
                for ic in range(n_ic):
                    i0 = ic * IC
                    isl = slice(i0, i0 + IC)
                    pv = pvp.tile([128, IC], F32, tag="pv")
                    hold = {}
                    for j in range(n_tt):
                        jsl = slice(j * 128, (j + 1) * 128)
                        # S matmuls: heads 2,3 first (their PSUM banks have
                        # the tightest turnaround), then 0,1; all adjacent.
                        s_c = scp.tile([128, IC], F32, tag="c", name="s_c")
                        s_d = sdp.tile([128, IC], F32, tag="d", name="s_d")
                        s_ab = sabp.tile([128, 2, IC], F32, tag="ab",
                                         name="s_ab")
                        for h, dst in ((2, s_c[:, 0:IC]), (3, s_d[:, 0:IC]),
                                       (0, s_ab[:, 0, 0:IC]),
                                       (1, s_ab[:, 1, 0:IC])):
                            hp = slice(32 * h, 32 * h + HS)
                            nc.tensor.matmul(
                                dst,
                                lhsT=qT[b][hp, jsl],
                                rhs=kT[b][hp, isl],
                                start=True, stop=True,
                                tile_position=(32 * h, 0))
                        # exps: DVE fast-exp head 2 (+3 odd j); ACT exact
                        # for 0,1 (+3 even j -- load balance ACT vs DVE)
                        ec = ecdp.tile([128, IC], I16, tag="ec", name="ec")
                        nc.vector.tensor_scalar(
                            out=ec, in0=s_c, scalar1=SCH_A, scalar2=SCH_B,
                            op0=OP.mult, op1=OP.add)
                        if j % 2 == 0:
                            ed = ecdp.tile([128, IC], BF16, tag="eda",
                                           name="eda")
                            nc.scalar.activation(out=ed, in_=s_d, func=AF.Exp)
                            ed_src = ed[:]
                        else:
                            ed = ecdp.tile([128, IC], I16, tag="ed", name="ed")
                            nc.vector.tensor_scalar(
                                out=ed, in0=s_d, scalar1=SCH_A, scalar2=SCH_B,
                                op0=OP.mult, op1=OP.add)
                            ed_src = ed[:].bitcast(BF16)
                        eab = eabp.tile([128, 2, IC], BF16, tag="eab",
                                        name="eab")
                        nc.scalar.activation(out=eab[:, :, 0:IC],
                                             in_=s_ab[:, :, 0:IC],
                                             func=AF.Exp)
                        # PV for previous j (its e tiles are long since done)
                        if prev_pv[0] is not None:
                            prev_pv[0]()
                        prev_pv[0] = (
                            lambda b=b, j=j, eab=eab, ec=ec, ed_src=ed_src,
                                   pv=pv, hold=hold:
                            emit_pv(b, j, eab, ec, ed_src, pv, hold))
                        # drip deferred work into the stream
                        if c_queue:
                            c_queue.pop(0)()
                        elif a_queue:
                            a_queue.pop(0)()
                        gstep[0] += 1
                    last_chunk = (b == b_loc - 1 and ic == n_ic - 1)
                    crings = ([(mp, "m"), (sabp, "ab"), (scp, "c"), (sdp, "d")]
                              if last_chunk else None)
                    c_queue.extend(make_stage_c_pieces(b, i0, hold,
                                                       rings=crings))
            prev_pv[0]()
            while c_queue:
                c_queue.pop(0)()

    if split_waits:
        _split_multiwaits(nc)
    return nc


def _split_multiwaits(nc):
    """walrus codegen in this container encodes a limited number of sem
    waits per instruction (1 for Drain, 2 for compute ops); spill extras
    onto preceding NOPs on the same engine. DMA copies are left alone —
    their waits ride in the DGE descriptor."""
    for func in nc.m.functions:
        for bb in func.blocks:
            insts = list(bb.instructions)
            out, changed = [], False
            for ins in insts:
                si = ins.sync_info
                maxw = 1
                if (maxw is not None and si is not None and si.on_wait
                        and len(si.on_wait) > maxw):
                    waits = list(si.on_wait)
                    for k, w in enumerate(waits[:-maxw]):
                        nop = mybir.InstNoOp(
                            name=f"{ins.name}-wsplit{k}",
                            sync_info=mybir.SyncInfo(on_wait=[w], on_update=[]),
                            bass_nofuse=True, engine=ins.engine)
                        try:
                            nc.register_instruction(nop, overwrite=True)
                        except Exception:
                            pass
                        out.append(nop)
                    si.on_wait = waits[-maxw:]
                    changed = True
                out.append(ins)
            if changed:
                bb.instructions = out


_NC_CACHE = {}


def kernel(**inputs):
    from concourse.bass_utils import run_bass_kernel_spmd

    x = np.ascontiguousarray(np.asarray(inputs["x"], dtype=np.float32))
    b_full = x.shape[0]
    n_cores = N_CORES
    b_loc = b_full // n_cores

    key = (b_loc, x.shape[1])
    if key not in _NC_CACHE:
        _NC_CACHE[key] = build_kernel(b_loc, x.shape[1])
    nc = _NC_CACHE[key]

    weights = {k: np.ascontiguousarray(np.asarray(inputs[k], dtype=np.float32))
               for k in ("Wq", "Wk", "Wv", "Wp", "bp", "W1", "b1", "W2", "b2",
                         "g1", "be1", "g2", "be2")}
    in_maps = []
    for c in range(n_cores):
        m = {"x": x[c * b_loc : (c + 1) * b_loc]}
        m.update(weights)
        in_maps.append(m)

    res = run_bass_kernel_spmd(nc, in_maps, core_ids=list(range(n_cores)))
    out = np.concatenate([r["out"] for r in res.results], axis=0)
    return out


# revision 41
# speedup vs baseline: 1.0453x; 1.0109x over previous
"""Trainium2 Bass kernel for a small dense transformer block (v2).

Model (per reference):
  x   : [B, T, D]  B=16, T=2048, D=40, H=4 heads, hs=10
  ln1 -> per-head q/k/v -> scores = k @ q^T (softmax over q index) -> out @ Wp
  residual (on ln1(x)) -> ln2 -> FFN(relu) -> residual (on ln2 output)

Sharding: data-parallel over batch, 2 batches per core across 8 cores.

v2 design notes (vs v1):
  - q and k packed in ONE bf16 buffer qkT [128, T]: head h has q at
    partitions 32h..32h+9 and k at partitions 32h+16..32h+25. One stage-A
    matmul per 512 tokens produces all of q+k for all heads; S matmuls
    become bf16 (1 cyc/row vs ~1.5-4 for fp32/f32r).
  - 4 S matmuls per j-tile emitted adjacently to 4 distinct PSUM banks
    (row-group tiling) so they can overlap in the PE.
  - exp split across engines: heads 0,1 via exact ACT exp (bf16 out),
    heads 2,3 via a Schraudolph-style fast exp on the DVE: one
    tensor_scalar producing int16 bf16-bit-patterns, bitcast to bf16 for
    the PV matmul. End-to-end rel err ~3e-3 (gate 2e-2).
  - v has an extra "ones" feature via a ones row in xnT (partition 40)
    and a ones entry in the packed v weights, giving the softmax
    denominator for free in the PV accumulation (as v1, minus the
    per-tile ones writes).
  - LN2 mean/rstd broadcast fused into ONE matmul ([2,512] f32r moving,
    [2,80] one-hot stationary -> [80,512]: rows 0:40 mu, 40:80 rstd).
  - reciprocal -> reciprocal_approx_fast (5x).
  - elementwise ops distributed: gpsimd takes SBUF-only ops (on, sq, x2,
    xn, msq), ACT takes PSUM-reading copies/activations it has slack for
    (pv_sb, mus, lnv2, rsd, hs-relu), DVE keeps the rest.
"""

import sys
from contextlib import ExitStack

for _p in ("/opt/trn_rl_repo",):
    if _p not in sys.path:
        sys.path.insert(0, _p)

import numpy as np

import concourse.bass as bass
import concourse.tile as tile
from concourse import mybir
from concourse.masks import make_identity

B_FULL = 16
N_CORES = 8
B_LOC = B_FULL // N_CORES
T = 2048
D = 40
H = 4
HS = 10
LN_EPS = 1e-5

F32 = mybir.dt.float32
F32R = mybir.dt.float32r
BF16 = mybir.dt.bfloat16
I16 = mybir.dt.int16
AF = mybir.ActivationFunctionType
OP = mybir.AluOpType

# Schraudolph fast-exp constants for bf16 bit patterns via int16:
#   bits = round(s * 128/ln2 + (127*128 - C))
SCH_A = 128.0 / float(np.log(2.0))
SCH_B = 127.0 * 128.0 - 5.5


def build_kernel(b_loc=B_LOC, t_len=T, split_waits=True):
    nc = bass.Bass("TRN2", target_bir_lowering=False)

    x_d = nc.dram_tensor("x", [b_loc, t_len, D], F32, kind="ExternalInput")
    wq_d = nc.dram_tensor("Wq", [H, D, HS], F32, kind="ExternalInput")
    wk_d = nc.dram_tensor("Wk", [H, D, HS], F32, kind="ExternalInput")
    wv_d = nc.dram_tensor("Wv", [H, D, HS], F32, kind="ExternalInput")
    wp_d = nc.dram_tensor("Wp", [D, D], F32, kind="ExternalInput")
    bp_d = nc.dram_tensor("bp", [D], F32, kind="ExternalInput")
    w1_d = nc.dram_tensor("W1", [D, D], F32, kind="ExternalInput")
    b1_d = nc.dram_tensor("b1", [D], F32, kind="ExternalInput")
    w2_d = nc.dram_tensor("W2", [D, D], F32, kind="ExternalInput")
    b2_d = nc.dram_tensor("b2", [D], F32, kind="ExternalInput")
    g1_d = nc.dram_tensor("g1", [D], F32, kind="ExternalInput")
    be1_d = nc.dram_tensor("be1", [D], F32, kind="ExternalInput")
    g2_d = nc.dram_tensor("g2", [D], F32, kind="ExternalInput")
    be2_d = nc.dram_tensor("be2", [D], F32, kind="ExternalInput")
    out_d = nc.dram_tensor("out", [b_loc, t_len, D], F32, kind="ExternalOutput")

    n_tt = t_len // 128                     # token tiles
    IC = 512 if t_len % 512 == 0 else t_len
    n_ic = t_len // IC
    NMA = min(512, t_len)                   # stage-A projection chunk

    with tile.TileContext(nc) as tc, ExitStack() as ctx:
        consts = ctx.enter_context(tc.tile_pool(name="consts", bufs=1))

        iden = consts.tile([128, 128], F32)
        make_identity(nc, iden)

        eps128 = consts.tile([128, 1], F32)
        nc.vector.memset(eps128, LN_EPS)

        # --- q/k projection weights: [40, 128] f32r, col 32h+e = W[h, :, e]
        def load_wqk(w_dram, name):
            t_ = consts.tile([D, 128], F32, tag=name)
            nc.vector.memset(t_, 0.0)
            for h in range(H):
                nc.gpsimd.dma_start(out=t_[:, 32 * h : 32 * h + HS],
                                    in_=w_dram[h])
            t_r = consts.tile([D, 128], F32R, tag=name + "r")
            nc.vector.tensor_copy(out=t_r, in_=t_)
            return t_r

        wqT_r = load_wqk(wq_d, "wqT")
        wkT_r = load_wqk(wk_d, "wkT")

        ones_row_t = consts.tile([1, 128], F32)
        nc.vector.memset(ones_row_t, 1.0)
        ones_row_w = consts.tile([1, t_len], F32)
        nc.vector.memset(ones_row_w, 1.0)

        # --- v weights [41, 128]: col 32h+e = Wv[h,:,e]; the extra
        #     row 40 has 1.0 at cols 32h+10 (ones feature -> softmax Z)
        wvf = consts.tile([D + 1, 128], F32)
        nc.vector.memset(wvf, 0.0)
        for h in range(H):
            nc.gpsimd.dma_start(out=wvf[0:D, 32 * h : 32 * h + HS],
                                in_=wv_d[h])
            nc.gpsimd.dma_start(
                out=wvf[D : D + 1, 32 * h + HS : 32 * h + HS + 1],
                in_=ones_row_t[0:1, 0:1])
        wv128e = consts.tile([D + 1, 128], F32R)
        nc.vector.tensor_copy(out=wv128e, in_=wvf)

        # Wp packed [128, 40]: row 32h+e = Wp[10h+e, :]; other rows zero
        wpp = consts.tile([128, D], F32)
        nc.vector.memset(wpp, 0.0)
        for h in range(H):
            nc.scalar.dma_start(
                out=wpp[32 * h : 32 * h + HS, :],
                in_=wp_d[HS * h : HS * h + HS, :],
            )
        wpp_r = consts.tile([128, D], F32R)
        nc.vector.tensor_copy(out=wpp_r, in_=wpp)

        w1s = consts.tile([D, D], F32)
        nc.scalar.dma_start(out=w1s, in_=w1_d[:])
        w1r = consts.tile([D, D], F32R)
        nc.vector.tensor_copy(out=w1r, in_=w1s)
        w2s = consts.tile([D, D], F32)
        nc.scalar.dma_start(out=w2s, in_=w2_d[:])
        w2r = consts.tile([D, D], F32R)
        nc.vector.tensor_copy(out=w2r, in_=w2s)

        def load_col(v_dram, name):
            t_ = consts.tile([D, 1], F32, tag=name)
            nc.gpsimd.dma_start(out=t_, in_=v_dram[:].unsqueeze(1))
            return t_

        bpc = load_col(bp_d, "bpc")
        b1c = load_col(b1_d, "b1c")
        b2c = load_col(b2_d, "b2c")
        g1c = load_col(g1_d, "g1c")
        be1c = load_col(be1_d, "be1c")
        g2c = load_col(g2_d, "g2c")
        be2c = load_col(be2_d, "be2c")

        # sel [128,128]: row 32h+10 has ones in cols 32h..32h+31 -> Z broadcast
        sel = consts.tile([128, 128], F32)
        nc.gpsimd.memset(sel, 0.0)
        nc.gpsimd.affine_select(
            out=sel[:].rearrange("p (g w) -> p g w", w=32),
            in_=sel[:].rearrange("p (g w) -> p g w", w=32),
            compare_op=OP.not_equal,
            fill=1.0,
            base=-HS,
            pattern=[[-32, 4], [0, 32]],
            channel_multiplier=1,
        )
        sel_r = consts.tile([128, 128], F32R)
        nc.vector.tensor_copy(out=sel_r, in_=sel)

        # mr_sel [2, 104]: row0 = 1 at cols 0:40, row1 = 1 at cols 64:104
        # (rstd broadcast lands at partitions 64:104 -- 32-aligned reads)
        MR2 = 64 + D
        mr_sel = consts.tile([2, MR2], F32)
        nc.vector.memset(mr_sel, 0.0)
        nc.vector.memset(mr_sel[0:1, 0:D], 1.0)
        nc.scalar.dma_start(out=mr_sel[1:2, 64:MR2],
                            in_=ones_row_t[0:1, 0:D])
        mr_sel_r = consts.tile([2, MR2], F32R)
        nc.vector.tensor_copy(out=mr_sel_r, in_=mr_sel)

        # ones/D column [40,1] for LN2 mean matmuls
        onesD_f = consts.tile([D, 1], F32)
        nc.vector.memset(onesD_f, 1.0 / D)
        onesD = consts.tile([D, 1], F32R)
        nc.vector.tensor_copy(out=onesD, in_=onesD_f)

        # ---------------- per-batch persistent SBUF ----------------
        persist = ctx.enter_context(tc.tile_pool(name="persist", bufs=1))
        xnT = [persist.tile([D + 1, t_len], F32R, tag=f"xnT{b}", name=f"xnT{b}")
               for b in range(b_loc)]
        qT = [persist.tile([128, t_len], BF16, tag=f"qT{b}", name=f"qT{b}")
              for b in range(b_loc)]
        kT = [persist.tile([128, t_len], BF16, tag=f"kT{b}", name=f"kT{b}")
              for b in range(b_loc)]
        vA = [persist.tile([128, n_tt, 128], BF16, tag=f"vA{b}", name=f"vA{b}")
              for b in range(b_loc)]

        # stage-A working pools
        sbA = ctx.enter_context(tc.tile_pool(name="sbA", bufs=4))
        xtp = ctx.enter_context(tc.tile_pool(name="xtp", bufs=n_tt + 1))

        with (
            tc.tile_pool(name="sab", bufs=2, space="PSUM") as sabp,
            tc.tile_pool(name="sc", bufs=1, space="PSUM") as scp,
            tc.tile_pool(name="sd", bufs=1, space="PSUM") as sdp,
            tc.tile_pool(name="pvp", bufs=1, space="PSUM") as pvp,
            tc.tile_pool(name="misc", bufs=1, space="PSUM") as mp,
            tc.tile_pool(name="eab", bufs=3) as eabp,
            tc.tile_pool(name="ecd", bufs=2) as ecdp,
            tc.tile_pool(name="sbC", bufs=2) as sC,
            tc.tile_pool(name="outp", bufs=4) as op_,
        ):
            # ================= stage A =================
            def emit_stage_a_slices(b, prologue, prefetch=None):
                mv = persist.tile([128, n_tt, 2], F32, tag=f"mv{b}",
                                  name=f"mv{b}")
                rstd = persist.tile([128, n_tt], F32, tag=f"rstd{b}",
                                    name=f"rstd{b}")
                GRP = min(4, n_tt)
                slices = []
                if prologue:
                    # before attention starts, all attention PSUM banks are
                    # free: rotate across them for deep pipelining
                    rings = [(sabp, "ab"), (scp, "c"), (sdp, "d"), (mp, "m")]
                else:
                    rings = [(mp, "m")]
                rk = [0]

                def psum(shape, name):
                    pool, tag = rings[rk[0] % len(rings)]
                    rk[0] += 1
                    return pool.tile(shape, F32, tag=tag, name=name)

                def ones_row(b=b):
                    nc.sync.dma_start(out=xnT[b][D : D + 1, :],
                                      in_=ones_row_w[:].bitcast(F32R))

                slices.append(ones_row)

                def ln_group(g0, b=b, mv=mv, rstd=rstd):
                    xts = {}
                    for t_i in range(g0, g0 + GRP):
                        if prefetch is not None:
                            xt = prefetch[t_i]
                        else:
                            xt = xtp.tile([128, D], F32, tag="xt", name="xt")
                            nc.sync.dma_start(
                                out=xt,
                                in_=x_d[b, t_i * 128 : (t_i + 1) * 128, :])
                        st6 = sbA.tile([128, 6], F32, tag="st6", name="st6")
                        nc.vector.bn_stats(out=st6, in_=xt)
                        nc.vector.bn_aggr(out=mv[:, t_i, :], in_=st6)
                        xts[t_i] = xt
                    lnv = sbA.tile([128, GRP], F32, tag="lnv", name="lnv")
                    nc.scalar.activation(out=lnv, in_=mv[:, g0 : g0 + GRP, 1],
                                         func=AF.Ln, bias=eps128, scale=1.0)
                    nc.scalar.activation(out=rstd[:, g0 : g0 + GRP], in_=lnv,
                                         func=AF.Exp, bias=0.0, scale=-0.5)
                    xn_eng = nc.vector if prologue else nc.gpsimd
                    for t_i in range(g0, g0 + GRP):
                        xn = sbA.tile([128, D], F32, tag="xn", name="xn")
                        xn_eng.tensor_scalar(
                            out=xn, in0=xts[t_i],
                            scalar1=mv[:, t_i, 0:1],
                            scalar2=rstd[:, t_i : t_i + 1],
                            op0=OP.subtract, op1=OP.mult)
                        tp = psum([D, 128], "tp")
                        nc.tensor.transpose(tp, xn, iden)
                        # fold ln1's gamma/beta into the PSUM evacuation
                        nc.vector.tensor_scalar(
                            out=xnT[b][0:D, t_i * 128 : (t_i + 1) * 128],
                            in0=tp, scalar1=g1c, scalar2=be1c,
                            op0=OP.mult, op1=OP.add)

                def qk_chunk(c, b=b):
                    sl = slice(c * NMA, (c + 1) * NMA)
                    qp = psum([128, NMA], "qp")
                    nc.tensor.matmul(qp, lhsT=wqT_r, rhs=xnT[b][0:D, sl],
                                     start=True, stop=True)
                    nc.vector.tensor_copy(out=qT[b][:, sl], in_=qp)
                    kp = psum([128, NMA], "kp")
                    nc.tensor.matmul(kp, lhsT=wkT_r, rhs=xnT[b][0:D, sl],
                                     start=True, stop=True)
                    nc.vector.tensor_copy(out=kT[b][:, sl], in_=kp)

                def v_pair(g0, b=b):
                    for t_i in range(g0, min(g0 + 2, n_tt)):
                        vp = psum([128, 128], "vp")
                        nc.tensor.matmul(
                            vp,
                            lhsT=xnT[b][:, t_i * 128 : (t_i + 1) * 128],
                            rhs=wv128e, start=True, stop=True)
                        nc.vector.tensor_copy(out=vA[b][:, t_i, :], in_=vp)

                for g0 in range(0, n_tt, GRP):
                    slices.append(lambda g0=g0: ln_group(g0))
                    slices.append(lambda c=g0 // GRP: qk_chunk(c))
                    slices.append(lambda g0=g0: v_pair(g0))
                    slices.append(lambda g0=g0 + 2: v_pair(g0))
                return slices, rings

            # ================= stage C =================
            def make_stage_c_pieces(b, i0, hold, rings=None, off=0, W=None,
                                    ring0=0, sfx=""):
                st = {}
                if W is None:
                    W = IC
                gsl = slice(i0 + off, i0 + off + W)
                if rings is None:
                    rings = [(mp, "m")]
                rk = [ring0]

                def cpsum(shape, name):
                    pool, tag = rings[rk[0] % len(rings)]
                    rk[0] += 1
                    return pool.tile(shape, F32, tag=tag, name=name)

                def p1():
                    pv_sb = hold["sb"][:, off : off + W]
                    st["pv_sb"] = pv_sb
                    zbc = cpsum([128, W], "zbc")
                    nc.tensor.matmul(zbc, lhsT=sel_r, rhs=pv_sb,
                                     start=True, stop=True)
                    # 1/Z = exp(-ln(Z)) on ACT (reads PSUM directly);
                    # keeps the expensive reciprocal off the DVE
                    lnz = sC.tile([128, W], F32, tag="lnz" + sfx,
                                  name="lnz")
                    nc.scalar.activation(out=lnz, in_=zbc, func=AF.Ln,
                                         bias=0.0, scale=1.0)
                    rbc = sC.tile([128, W], F32, tag="rbc" + sfx,
                                  name="rbc")
                    nc.scalar.activation(out=rbc, in_=lnz, func=AF.Exp,
                                         bias=0.0, scale=-1.0)
                    st["rbc"] = rbc

                def p2():
                    on = sC.tile([128, W], F32R, tag="on" + sfx, name="on")
                    nc.gpsimd.tensor_mul(out=on, in0=st["pv_sb"],
                                         in1=st["rbc"])
                    st["on"] = on

                def p3():
                    yp = cpsum([D, W], "yp")
                    nc.tensor.matmul(yp, lhsT=wpp_r, rhs=st["on"],
                                     start=True, stop=True)
                    x1 = sC.tile([D, W], F32R, tag="x1" + sfx, name="x1")
                    nc.vector.scalar_tensor_tensor(
                        out=x1, in0=yp, scalar=bpc, in1=xnT[b][0:D, gsl],
                        op0=OP.add, op1=OP.add)
                    st["x1"] = x1

                def p4():
                    sq = sC.tile([D, W], F32R, tag="sq" + sfx, name="sq")
                    nc.gpsimd.tensor_mul(out=sq, in0=st["x1"], in1=st["x1"])
                    st["sq"] = sq
                    mup = cpsum([1, W], "mup")
                    nc.tensor.matmul(mup, lhsT=onesD, rhs=st["x1"],
                                     start=True, stop=True)
                    mus = sC.tile([1, W], F32R, tag="mus" + sfx,
                                  name="mus")
                    nc.scalar.copy(out=mus, in_=mup)
                    mr = sC.tile([2, W], F32R, tag="mr" + sfx, name="mr")
                    nc.gpsimd.dma_start(out=mr[0:1, :], in_=mus[:])
                    st["mus"] = mus
                    st["mr"] = mr

                def p5():
                    m2p = cpsum([1, W], "m2p")
                    nc.tensor.matmul(m2p, lhsT=onesD, rhs=st["sq"],
                                     start=True, stop=True)
                    msq = sC.tile([1, W], F32, tag="msq" + sfx,
                                  name="msq")
                    nc.gpsimd.tensor_mul(out=msq, in0=st["mus"],
                                         in1=st["mus"])
                    var = sC.tile([1, W], F32, tag="var" + sfx,
                                  name="var")
                    nc.vector.tensor_sub(out=var, in0=m2p, in1=msq)
                    lnv2 = sC.tile([1, W], F32, tag="lnv2" + sfx,
                                  name="lnv2")
                    nc.scalar.activation(out=lnv2, in_=var, func=AF.Ln,
                                         bias=eps128[0:1, :], scale=1.0)
                    rsd = sC.tile([1, W], F32R, tag="rsd" + sfx,
                                  name="rsd")
                    nc.scalar.activation(out=rsd, in_=lnv2,
                                         func=AF.Exp, bias=0.0, scale=-0.5)
                    nc.gpsimd.dma_start(out=st["mr"][1:2, :], in_=rsd[:])

                def p6():
                    mrbc = cpsum([MR2, W], "mrbc")
                    nc.tensor.matmul(mrbc, lhsT=mr_sel_r, rhs=st["mr"],
                                     start=True, stop=True)
                    t1 = sC.tile([D, W], F32, tag="t1" + sfx, name="t1")
                    nc.vector.tensor_sub(out=t1, in0=st["x1"], in1=mrbc[0:D, :])
                    t2 = sC.tile([D, W], F32R, tag="t2" + sfx, name="t2")
                    nc.vector.tensor_mul(out=t2, in0=t1, in1=mrbc[64:MR2, :])
                    x2 = sC.tile([D, W], F32R, tag="x2" + sfx, name="x2")
                    nc.gpsimd.tensor_scalar(out=x2, in0=t2, scalar1=g2c,
                                            scalar2=be2c, op0=OP.mult,
                                            op1=OP.add)
                    st["x2"] = x2

                def p7():
                    hp = cpsum([D, W], "hp")
                    nc.tensor.matmul(hp, lhsT=w1r, rhs=st["x2"],
                                     start=True, stop=True)
                    hs = sC.tile([D, W], F32R, tag="hs" + sfx, name="hs")
                    nc.vector.tensor_scalar(
                        out=hs, in0=hp, scalar1=b1c, scalar2=0.0,
                        op0=OP.add, op1=OP.max)
                    st["hs"] = hs

                def p8():
                    y2p = cpsum([D, W], "y2p")
                    nc.tensor.matmul(y2p, lhsT=w2r, rhs=st["hs"],
                                     start=True, stop=True)
                    ob = sC.tile([D, W], F32, tag="ob" + sfx, name="ob")
                    nc.vector.scalar_tensor_tensor(
                        out=ob, in0=y2p, scalar=b2c, in1=st["x2"],
                        op0=OP.add, op1=OP.add)
                    st["ob"] = ob

                def out_piece(tt0):
                    for tt_i in range(tt0, min(tt0 + 2, W // 128)):
                        otp = cpsum([128, D], "otp")
                        nc.tensor.transpose(
                            otp, st["ob"][:, tt_i * 128 : (tt_i + 1) * 128],
                            iden[0:D, 0:D])
                        osb = op_.tile([128, D], F32, tag="osb", name="osb")
                        nc.vector.tensor_copy(out=osb, in_=otp)
                        t_glob = i0 + off + tt_i * 128
                        nc.sync.dma_start(
                            out=out_d[b, t_glob : t_glob + 128, :], in_=osb)

                return [p1, p2, p3, p4, p5, p6, p7, p8,
                        lambda: out_piece(0), lambda: out_piece(2)]

            # ================= attention main loop =================
            # prefetch ALL of batch 0's x tiles first thing on the sync
            # queue -- nothing else competes there at t=0
            b0_x = {}
            for t_i in range(n_tt):
                xt = xtp.tile([128, D], F32, tag="xt", name="xt")
                nc.sync.dma_start(
                    out=xt, in_=x_d[0, t_i * 128 : (t_i + 1) * 128, :])
                b0_x[t_i] = xt
            # emit only the first 512 tokens' worth of batch-0 stage A up
            # front (what chunk 0 needs to start); drip the rest into the
            # chunk-0 attention stream
            a0_slices, a0_rings = emit_stage_a_slices(0, prologue=True,
                                                      prefetch=b0_x)
            for f in a0_slices[:5]:
                f()
            a0_rings[:] = [(mp, "m")]
            a_queue = list(a0_slices[5:])
            for b2 in range(1, b_loc):
                s2, _ = emit_stage_a_slices(b2, prologue=False)
                a_queue.extend(s2)

            c_queue = []
            prev_pv = [None]

            def emit_pv(b, j, eab, ec, ed_src, pv, hold):
                srcs = [eab[:, 0, :], eab[:, 1, :],
                        ec[:].bitcast(BF16), ed_src]
                for h in range(H):
                    nc.tensor.matmul(
                        pv[32 * h : 32 * h + 32, :],
                        lhsT=vA[b][:, j, 32 * h : 32 * h + 32],
                        rhs=srcs[h],
                        start=(j == 0), stop=(j == n_tt - 1),
                        skip_group_check=True,
                        tile_position=(0, 32 * h))
                if j == n_tt - 1:
                    pv_sb = sC.tile([128, IC], F32R, tag="pvsb", name="pv_sb")
                    nc.scalar.copy(out=pv_sb, in_=pv)
                    hold["sb"] = pv_sb

            gstep = [0]
            for b in range(b_loc):
                if b > 0:
                    while a_queue:
                        a_queue.pop(0)()
                for ic in range(n_ic):
                    i0 = ic * IC
                    isl = slice(i0, i0 + IC)
                    pv = pvp.tile([128, IC], F32, tag="pv")
                    hold = {}
                    for j in range(n_tt):
                        jsl = slice(j * 128, (j + 1) * 128)
                        # S matmuls: heads 2,3 first (their PSUM banks have
                        # the tightest turnaround), then 0,1; all adjacent.
                        s_c = scp.tile([128, IC], F32, tag="c", name="s_c")
                        s_d = sdp.tile([128, IC], F32, tag="d", name="s_d")
                        s_ab = sabp.tile([128, 2, IC], F32, tag="ab",
                                         name="s_ab")
                        for h, dst in ((2, s_c[:, 0:IC]), (3, s_d[:, 0:IC]),
                                       (0, s_ab[:, 0, 0:IC]),
                                       (1, s_ab[:, 1, 0:IC])):
                            hp = slice(32 * h, 32 * h + HS)
                            nc.tensor.matmul(
                                dst,
                                lhsT=qT[b][hp, jsl],
                                rhs=kT[b][hp, isl],
                                start=True, stop=True,
                                tile_position=(32 * h, 0))
                        # exps: DVE fast-exp head 2 (+3 odd j); ACT exact
                        # for 0,1 (+3 even j -- load balance ACT vs DVE)
                        ec = ecdp.tile([128, IC], I16, tag="ec", name="ec")
                        nc.vector.tensor_scalar(
                            out=ec, in0=s_c, scalar1=SCH_A, scalar2=SCH_B,
                            op0=OP.mult, op1=OP.add)
                        if j % 2 == 0:
                            ed = ecdp.tile([128, IC], BF16, tag="eda",
                                           name="eda")
                            nc.scalar.activation(out=ed, in_=s_d, func=AF.Exp)
                            ed_src = ed[:]
                        else:
                            ed = ecdp.tile([128, IC], I16, tag="ed", name="ed")
                            nc.vector.tensor_scalar(
                                out=ed, in0=s_d, scalar1=SCH_A, scalar2=SCH_B,
                                op0=OP.mult, op1=OP.add)
                            ed_src = ed[:].bitcast(BF16)
                        eab = eabp.tile([128, 2, IC], BF16, tag="eab",
                                        name="eab")
                        nc.scalar.activation(out=eab[:, :, 0:IC],
                                             in_=s_ab[:, :, 0:IC],
                                             func=AF.Exp)
                        # PV for previous j (its e tiles are long since done)
                        if prev_pv[0] is not None:
                            prev_pv[0]()
                        prev_pv[0] = (
                            lambda b=b, j=j, eab=eab, ec=ec, ed_src=ed_src,
                                   pv=pv, hold=hold:
                            emit_pv(b, j, eab, ec, ed_src, pv, hold))
                        # drip deferred work into the stream
                        if c_queue:
                            c_queue.pop(0)()
                        elif a_queue:
                            a_queue.pop(0)()
                        gstep[0] += 1
                    last_chunk = (b == b_loc - 1 and ic == n_ic - 1)
                    if last_chunk:
                        # 4 independent 128-wide chains on rotated psum
                        # rings so the final drain pipelines instead of
                        # serializing one long dependency chain
                        crings = [(mp, "m"), (sabp, "ab"), (scp, "c"),
                                  (sdp, "d")]
                        chains = [
                            make_stage_c_pieces(b, i0, hold, rings=crings,
                                                off=128 * k, W=128,
                                                ring0=k, sfx=f"_{k}")
                            for k in range(4)
                        ]
                        for step in range(len(chains[0])):
                            for ch in chains:
                                c_queue.append(ch[step])
                    else:
                        c_queue.extend(make_stage_c_pieces(b, i0, hold))
            prev_pv[0]()
            while c_queue:
                c_queue.pop(0)()

    if split_waits:
        _split_multiwaits(nc)
    return nc


def _split_multiwaits(nc):
    """walrus codegen in this container encodes a limited number of sem
    waits per instruction (1 for Drain, 2 for compute ops); spill extras
    onto preceding NOPs on the same engine. DMA copies are left alone —
    their waits ride in the DGE descriptor."""
    for func in nc.m.functions:
        for bb in func.blocks:
            insts = list(bb.instructions)
            out, changed = [], False
            for ins in insts:
                si = ins.sync_info
                maxw = 1
                if (maxw is not None and si is not None and si.on_wait
                        and len(si.on_wait) > maxw):
                    waits = list(si.on_wait)
                    for k, w in enumerate(waits[:-maxw]):
                        nop = mybir.InstNoOp(
                            name=f"{ins.name}-wsplit{k}",
                            sync_info=mybir.SyncInfo(on_wait=[w], on_update=[]),
                            bass_nofuse=True, engine=ins.engine)
                        try:
                            nc.register_instruction(nop, overwrite=True)
                        except Exception:
                            pass
                        out.append(nop)
                    si.on_wait = waits[-maxw:]
                    changed = True
                out.append(ins)
            if changed:
                bb.instructions = out


_NC_CACHE = {}


def kernel(**inputs):
    from concourse.bass_utils import run_bass_kernel_spmd

    x = np.ascontiguousarray(np.asarray(inputs["x"], dtype=np.float32))
    b_full = x.shape[0]
    n_cores = N_CORES
    b_loc = b_full // n_cores

    key = (b_loc, x.shape[1])
    if key not in _NC_CACHE:
        _NC_CACHE[key] = build_kernel(b_loc, x.shape[1])
    nc = _NC_CACHE[key]

    weights = {k: np.ascontiguousarray(np.asarray(inputs[k], dtype=np.float32))
               for k in ("Wq", "Wk", "Wv", "Wp", "bp", "W1", "b1", "W2", "b2",
                         "g1", "be1", "g2", "be2")}
    in_maps = []
    for c in range(n_cores):
        m = {"x": x[c * b_loc : (c + 1) * b_loc]}
        m.update(weights)
        in_maps.append(m)

    res = run_bass_kernel_spmd(nc, in_maps, core_ids=list(range(n_cores)))
    out = np.concatenate([r["out"] for r in res.results], axis=0)
    return out
